# revision 43
# baseline (speedup 1.0000x reference)
"""CARAFE kernel for 8x Trainium2 NeuronCores.

Sharding: core = b*4 + q  (b in [0,2) batch, q in [0,4) H-slice).
Each core handles source rows [16q, 16q+16) of batch b and produces
output rows [32q, 32q+32) (x2 upsampling).

Pipeline per core (bf16 compute, fp32 PSUM/softmax stats):
  comp 1x1 conv (PE, BN folded, ReLU on ACT)
  enc 3x3 conv transposed (PE; psum = [pixels, 100ch]) + bias via K=1 matmul
  softmax over k=25 per parity group (ACT exp, DVE reduce/recip, one
    broadcast tensor_tensor for normalization)
  reassembly: 25-tap per-pixel weighted sum, split across three engine
    channels running in parallel:
      - Pool builds diagonal weight matrices (identity * per-partition
        scalar, 107ns) feeding PE matmuls that accumulate in PSUM
        (107ns each); ACT evacuates PSUM -> bf16 accs
      - DVE runs scalar_tensor_tensor FMA quanta directly (327ns each)
"""

import sys

if "/opt/trn_rl_repo" not in sys.path:
    sys.path.insert(0, "/opt/trn_rl_repo")

import numpy as np

try:
    import ml_dtypes

    BF16 = ml_dtypes.bfloat16
except ImportError:  # pragma: no cover
    BF16 = np.float32

SCALE = 2
K_UP = 5
EPS = 1e-5
B, C, H, W = 2, 256, 64, 64
CM = 64
NK = 100
NCORES = 8
QH = 16           # source rows per core
LH = QH + 4       # local x rows incl halo 2 each side
NW1R = 18         # W1 rows kept: x-local rows [1,19)

# (par, lam) pairs handled by the DVE STT channel; the rest go to the
# pool->PE diagonal-matmul channel.
DVE_PAIRS = [(3, l) for l in range(8)] + [(2, 7)]
POOL_PAIRS = []

_compiled = {}


def _build_nc():
    import concourse.bacc as bacc
    import concourse.bass as bass
    import concourse.mybir as mybir
    import concourse.tile as tile

    f32 = mybir.dt.float32
    bf16 = mybir.dt.bfloat16
    nc = bacc.Bacc("TRN2", target_bir_lowering=False, debug=False)

    # ---- DRAM I/O ----
    x_nat_d = nc.dram_tensor("x_nat", [128, 2, NW1R, W], bf16, kind="ExternalInput")
    xs_d = {}
    for p in range(2):
        for kj in range(K_UP):
            xs_d[(kj, p)] = nc.dram_tensor(
                f"xs_{kj}_{p}", [128, 10, C], bf16, kind="ExternalInput"
            )
    comp_lhsT_d = nc.dram_tensor("comp_lhsT", [128, 2, CM], bf16, kind="ExternalInput")
    comp_bias_d = nc.dram_tensor("comp_bias", [CM, 1], f32, kind="ExternalInput")
    enc_pair_d = nc.dram_tensor("enc_pair", [128, 3, NK], bf16, kind="ExternalInput")
    enc_single_d = nc.dram_tensor("enc_single", [CM, 3, NK], bf16, kind="ExternalInput")
    enc_bias_d = nc.dram_tensor("enc_bias", [1, NK], bf16, kind="ExternalInput")
    rmask_d = nc.dram_tensor("rmask", [CM, NW1R], bf16, kind="ExternalInput")
    iden_d = nc.dram_tensor("iden", [128, 128], bf16, kind="ExternalInput")
    out_d = nc.dram_tensor("out", [4, 128, 8, C], bf16, kind="ExternalOutput")

    taps = [(ki, kj) for ki in (0, 2, 4, 1, 3) for kj in range(K_UP)]
    pe_pairs = [(par, lam) for par in range(4) for lam in range(8)
                if (par, lam) not in DVE_PAIRS]

    with tile.TileContext(nc) as tc:
        with (
            tc.tile_pool(name="consts", bufs=1) as consts,
            tc.tile_pool(name="big", bufs=1) as big,
            tc.tile_pool(name="work", bufs=2) as work,
            tc.tile_pool(name="dgs", bufs=1) as dgs,
        ):
            # ---- load constants; comp weights + x_nat first so the
            # comp->enc->softmax chain starts ASAP ----
            comp_lhsT = consts.tile([128, 2, CM], bf16, tag="comp_lhsT")
            nc.sync.dma_start(comp_lhsT, comp_lhsT_d[:])
            x_nat = big.tile([128, 2, NW1R, W], bf16, tag="x_nat")
            nc.sync.dma_start(x_nat[:, 0], x_nat_d[:, 0])
            nc.sync.dma_start(x_nat[:, 1], x_nat_d[:, 1])
            comp_bias = consts.tile([CM, 1], f32, tag="comp_bias")
            nc.sync.dma_start(comp_bias, comp_bias_d[:])
            rmask = consts.tile([CM, NW1R], bf16, tag="rmask")
            nc.sync.dma_start(rmask, rmask_d[:])
            enc_pair = consts.tile([128, 3, NK], bf16, tag="enc_pair")
            nc.sync.dma_start(enc_pair, enc_pair_d[:])
            enc_single = consts.tile([CM, 3, NK], bf16, tag="enc_single")
            nc.sync.dma_start(enc_single, enc_single_d[:])
            enc_bias = consts.tile([1, NK], bf16, tag="enc_bias")
            nc.sync.dma_start(enc_bias, enc_bias_d[:])
            iden = consts.tile([128, 128], bf16, tag="iden")
            nc.sync.dma_start(iden, iden_d[:])
            ones_row = consts.tile([1, 128], bf16, tag="ones_row")
            nc.vector.memset(ones_row, 1.0)
            # xs in tap-consumption order: p=0 tensors (ki even) first
            xs_t = {}
            for p in range(2):
                for kj in range(K_UP):
                    t = big.tile([128, 10, C], bf16, tag=f"xs_{kj}_{p}")
                    nc.sync.dma_start(t, xs_d[(kj, p)][:])
                    xs_t[(kj, p)] = t

            # ---- comp 1x1 conv: psum[64, 18*64] = comp_w' @ x ----
            pcomp = tc.alloc_tile_pool(name="pcomp", bufs=1, space="PSUM")
            penc = tc.alloc_tile_pool(name="penc", bufs=2, space="PSUM")  # 2 slots: pipeline lam
            psum_c = pcomp.tile([CM, NW1R * W], f32, tag="psum_c")
            nchunks = [(0, 512), (512, 512), (1024, NW1R * W - 1024)]
            x_nat_f = x_nat.rearrange("p h r w -> p h (r w)")
            for h in range(2):
                for n0, nl in nchunks:
                    nc.tensor.matmul(
                        psum_c[:, n0 : n0 + nl],
                        lhsT=comp_lhsT[:, h, :],
                        rhs=x_nat_f[:, h, n0 : n0 + nl],
                        start=(h == 0),
                        stop=(h == 1),
                    )

            # ---- W1 (bf16) + row mask ----
            w1c = big.tile([CM, NW1R, W], bf16, tag="w1c")
            psum_c_v = psum_c.rearrange("p (r w) -> p r w", w=W)
            nc.scalar.activation(
                out=w1c,
                in_=psum_c_v,
                func=mybir.ActivationFunctionType.Relu,
                bias=comp_bias,
                scale=1.0,
            )
            rmask_b = bass.AP(
                tensor=rmask.tensor,
                offset=rmask.offset,
                ap=[list(rmask.ap[0]), list(rmask.ap[1]), [0, W]],
            )
            nc.gpsimd.tensor_tensor(
                out=w1c, in0=w1c, in1=rmask_b, op=mybir.AluOpType.mult
            )
            # w1main: rows 0-63 = W1 shifted right (left tap), rows 64-127 = center
            w1main = big.tile([128, NW1R, W], bf16, tag="w1main")
            nc.vector.memset(w1main[0:CM, :, 0:1], 0.0)
            nc.vector.tensor_copy(out=w1main[CM:128], in_=w1c)
            nc.vector.tensor_copy(
                out=w1main[0:CM, :, 1:W], in_=w1c[:, :, 0 : W - 1]
            )
            # w1sing: W1 shifted left (right tap)
            w1sing = big.tile([CM, NW1R, W], bf16, tag="w1sing")
            nc.vector.memset(w1sing[:, :, W - 1 : W], 0.0)
            nc.vector.tensor_copy(
                out=w1sing[:, :, 0 : W - 1], in_=w1c[:, :, 1:W]
            )
            w1main_f = w1main.rearrange("p r w -> p (r w)")
            w1sing_f = w1sing.rearrange("p r w -> p (r w)")

            # ---- enc 3x3 conv (transposed) + exp; the softmax division
            # is deferred into accumulator normalization ----
            E32 = big.tile([128, 8, NK], f32, tag="E32")
            S = work.tile([128, 8, 4], f32, tag="S")
            R = work.tile([128, 8, 4], f32, tag="R")
            E_v = E32.rearrange("p l (k q) -> p l q k", q=4)
            for lam in range(8):
                psum_e = penc.tile([128, NK], f32, tag="psum_e")
                for ty in range(3):
                    o = (2 * lam + ty) * W
                    nc.tensor.matmul(
                        psum_e,
                        lhsT=w1main_f[:, o : o + 2 * W],
                        rhs=enc_pair[:, ty, :],
                        start=(ty == 0),
                        stop=False,
                    )
                for ty in range(3):
                    o = (2 * lam + ty) * W
                    nc.tensor.matmul(
                        psum_e,
                        lhsT=w1sing_f[:, o : o + 2 * W],
                        rhs=enc_single[:, ty, :],
                        start=False,
                        stop=False,
                    )
                nc.tensor.matmul(
                    psum_e, lhsT=ones_row, rhs=enc_bias, start=False, stop=True
                )
                nc.scalar.activation(
                    out=E32[:, lam, :],
                    in_=psum_e,
                    func=mybir.ActivationFunctionType.Exp,
                )
                nc.vector.tensor_reduce(
                    out=S[:, lam], in_=E_v[:, lam],
                    axis=mybir.AxisListType.X, op=mybir.AluOpType.add,
                )
                nc.vector.reciprocal(out=R[:, lam], in_=S[:, lam])

            penc.release()
            pcomp.release()
            preass = tc.alloc_tile_pool(name="preass", bufs=1, space="PSUM")

            # ---- reassembly ----
            accs = [
                big.tile([128, 8, C], bf16, tag=f"acc{par}", name=f"acc{par}")
                for par in range(4)
            ]
            prods = [
                work.tile([128, C], bf16, tag=f"prod{j}", name=f"prod{j}")
                for j in range(4)
            ]

            def tap_in0(t, lam):
                ki, kj = taps[t]
                return xs_t[(kj, ki % 2)][:, lam + ki // 2, :]

            def tap_ch(t, par):
                ki, kj = taps[t]
                return (ki * K_UP + kj) * 4 + par

            # DVE channel: pairs (par=3, lam 0..7), tap-major; roughly
            # every other tap is ACT-assisted (ACT does the scale-mult,
            # DVE only the 2x bf16 add)
            dve_ops = []
            chan_pairs = ([("pool", p) for p in POOL_PAIRS]
                          + [("dve", p) for p in DVE_PAIRS])
            blk = 3
            for b0 in range(0, len(chan_pairs), blk):
                block = chan_pairs[b0 : b0 + blk]
                for t in range(len(taps)):
                    for kind, (par, lam) in block:
                        dve_ops.append((kind, t, par, lam))

            # PE channel: lam-major pair order so lam-0 quanta unblock
            # right after lam-0 softmax; waves of 8 psum slots
            pe_pair_list = [(par, lam) for lam in range(8) for par in range(4)
                            if (par, lam) not in DVE_PAIRS
                            and (par, lam) not in POOL_PAIRS]
            pe_ops = []
            wv = 6
            for w0 in range(0, len(pe_pair_list), wv):
                wave = pe_pair_list[w0 : w0 + wv]
                for t in range(len(taps)):
                    for wi, (par, lam) in enumerate(wave):
                        pe_ops.append(("mm", t, par, lam, w0 + wi))
                        if t == len(taps) - 1:
                            pe_ops.append(("evac", par, lam, w0 + wi))

            psum_tiles = {}
            diag_i = 0
            prod_i = 0
            di, pi = 0, 0
            while di < len(dve_ops) or pi < len(pe_ops):
                for _ in range(4):
                    if pi < len(pe_ops):
                        op = pe_ops[pi]
                        pi += 1
                        if op[0] == "mm":
                            _, t, par, lam, slot = op
                            if t == 0:
                                psum_tiles[slot] = preass.tile(
                                    [128, C], f32, tag=f"pr{slot % 8}",
                                    name=f"pr{slot % 8}",
                                )
                            dg = dgs.tile(
                                [128, 128], bf16, tag=f"dg{diag_i % 16}",
                                name=f"dg{diag_i % 16}",
                            )
                            diag_i += 1
                            ch = tap_ch(t, par)
                            nc.gpsimd.tensor_scalar_mul(
                                dg, iden, E32[:, lam, ch : ch + 1]
                            )
                            nc.tensor.matmul(
                                psum_tiles[slot],
                                lhsT=dg,
                                rhs=tap_in0(t, lam),
                                start=(t == 0),
                                stop=(t == len(taps) - 1),
                            )
                        else:
                            _, par, lam, slot = op
                            if slot % 3 != 2:
                                nc.scalar.activation(
                                    out=accs[par][:, lam, :],
                                    in_=psum_tiles[slot],
                                    func=mybir.ActivationFunctionType.Copy,
                                    scale=R[:, lam, par : par + 1],
                                )
                            else:
                                nc.vector.tensor_scalar_mul(
                                    accs[par][:, lam, :],
                                    psum_tiles[slot],
                                    R[:, lam, par : par + 1],
                                )
                            nc.sync.dma_start(
                                out_d[par, :, lam], accs[par][:, lam, :]
                            )
                if di < len(dve_ops):
                    kind, t, par, lam = dve_ops[di]
                    di += 1
                    in0 = tap_in0(t, lam)
                    ch = tap_ch(t, par)
                    acc = accs[par][:, lam, :]
                    sc = E32[:, lam, ch : ch + 1]
                    if kind == "pool":
                        nc.gpsimd.scalar_tensor_tensor(
                            out=acc,
                            in0=in0,
                            scalar=sc,
                            in1=in0 if t == 0 else acc,
                            op0=mybir.AluOpType.mult,
                            op1=mybir.AluOpType.bypass
                            if t == 0
                            else mybir.AluOpType.add,
                        )
                        if t == len(taps) - 1:
                            nc.sync.dma_start(
                                out_d[par, :, lam], accs[par][:, lam, :]
                            )
                        continue
                    if t > 0 and (t % 2 == 0 or t in (3, 9)):
                        # ACT-assisted: prod = w * x on ACT, acc += prod on DVE
                        prod = prods[prod_i % 4]
                        prod_i += 1
                        nc.scalar.activation(
                            out=prod, in_=in0,
                            func=mybir.ActivationFunctionType.Copy, scale=sc,
                        )
                        nc.vector.tensor_tensor(
                            out=acc, in0=acc, in1=prod, op=mybir.AluOpType.add
                        )
                    else:
                        nc.vector.scalar_tensor_tensor(
                            out=acc,
                            in0=in0,
                            scalar=sc,
                            in1=in0 if t == 0 else acc,
                            op0=mybir.AluOpType.mult,
                            op1=mybir.AluOpType.bypass
                            if t == 0
                            else mybir.AluOpType.add,
                        )
                    if t == len(taps) - 1:
                        nc.vector.tensor_scalar_mul(
                            accs[par][:, lam, :], accs[par][:, lam, :],
                            R[:, lam, par : par + 1],
                        )
                        nc.sync.dma_start(
                            out_d[par, :, lam], accs[par][:, lam, :]
                        )

            preass.release()

    nc.compile()
    return nc


def _host_inputs(x, comp_w, comp_gamma, comp_beta, comp_mean, comp_var,
                 enc_w, enc_gamma, enc_beta, enc_mean, enc_var):
    """Build the per-core input maps."""
    x = np.asarray(x, dtype=np.float32)
    scale_c = (np.asarray(comp_gamma) / np.sqrt(np.asarray(comp_var) + EPS)).astype(
        np.float32
    )
    bias_c = (np.asarray(comp_beta) - np.asarray(comp_mean) * scale_c).astype(
        np.float32
    )
    wp = np.asarray(comp_w)[:, :, 0, 0].astype(np.float32) * scale_c[:, None]  # [64,256]
    comp_lhsT = np.ascontiguousarray(
        wp.T.reshape(2, 128, CM).transpose(1, 0, 2)
    ).astype(BF16)
    comp_bias = bias_c.reshape(CM, 1)

    scale_e = (np.asarray(enc_gamma) / np.sqrt(np.asarray(enc_var) + EPS)).astype(
        np.float32
    )
    bias_e = (np.asarray(enc_beta) - np.asarray(enc_mean) * scale_e).astype(np.float32)
    ew = np.asarray(enc_w).astype(np.float32) * scale_e[:, None, None, None]  # [100,64,3,3]
    enc_pair = np.zeros((128, 3, NK), np.float32)
    enc_single = np.zeros((CM, 3, NK), np.float32)
    for ty in range(3):
        enc_pair[0:CM, ty] = ew[:, :, ty, 0].T
        enc_pair[CM:128, ty] = ew[:, :, ty, 1].T
        enc_single[:, ty] = ew[:, :, ty, 2].T
    enc_bias_a = bias_e.reshape(1, NK).astype(BF16)

    iden = np.eye(128, dtype=np.float32).astype(BF16)

    in_maps = []
    for core in range(NCORES):
        b, q = core // 4, core % 4
        # padded x slab: rows global [16q-2, 16q+19) -> 21 rows, cols padded +2
        xpad = np.zeros((C, LH + 1, W + 4), np.float32)
        g0, g1 = 16 * q - 2, 16 * q + 19
        s0, s1 = max(g0, 0), min(g1, H)
        xpad[:, s0 - g0 : s1 - g0, 2 : W + 2] = x[b, :, s0:s1, :]

        x_nat = np.ascontiguousarray(
            xpad[:, 1 : 1 + NW1R, 2 : W + 2].reshape(2, 128, NW1R, W).transpose(
                1, 0, 2, 3
            )
        ).astype(BF16)

        m = {"x_nat": x_nat, "comp_lhsT": comp_lhsT, "comp_bias": comp_bias,
             "enc_pair": enc_pair.astype(BF16),
             "enc_single": enc_single.astype(BF16),
             "enc_bias": enc_bias_a, "iden": iden}
        for kj in range(K_UP):
            for p in range(2):
                xv = xpad[:, p : p + 20, kj : kj + W]  # [C, 20, 64]
                arr = xv.reshape(C, 10, 2, W).transpose(2, 3, 1, 0)  # [2,64,10,C]
                m[f"xs_{kj}_{p}"] = np.ascontiguousarray(
                    arr.reshape(128, 10, C)
                ).astype(BF16)
        ridx = np.arange(NW1R)
        grows = 16 * q - 1 + ridx
        m["rmask"] = np.ascontiguousarray(
            np.broadcast_to(
                ((grows >= 0) & (grows < H)).astype(np.float32), (CM, NW1R)
            )
        ).astype(BF16)
        in_maps.append(m)
    return in_maps


def _assemble(results):
    out = np.zeros((B, C, H * SCALE, W * SCALE), np.float32)
    for core in range(NCORES):
        b, q = core // 4, core % 4
        arr = np.asarray(results[core]["out"]).astype(np.float32)  # [4,128,8,256]
        a = arr.reshape(2, 2, 2, 64, 8, C)  # [di, dj, rho, j, lam, c]
        blk = a.transpose(5, 4, 2, 0, 3, 1).reshape(C, 32, 128)
        out[b, :, 32 * q : 32 * q + 32, :] = blk
    return out


def kernel(**inputs):
    from concourse.bass_utils import run_bass_kernel_spmd

    if "nc" not in _compiled:
        _compiled["nc"] = _build_nc()
    nc = _compiled["nc"]
    in_maps = _host_inputs(**inputs)
    res = run_bass_kernel_spmd(nc, in_maps, core_ids=list(range(NCORES)))
    return _assemble(res.results)


# revision 52
# speedup vs baseline: 1.0042x; 1.0042x over previous
"""CARAFE kernel for 8x Trainium2 NeuronCores.

Sharding: core = b*4 + q  (b in [0,2) batch, q in [0,4) H-slice).
Each core handles source rows [16q, 16q+16) of batch b and produces
output rows [32q, 32q+32) (x2 upsampling).

Pipeline per core (bf16 compute, fp32 PSUM/softmax stats):
  comp 1x1 conv (PE, BN folded, ReLU on ACT)
  enc 3x3 conv transposed (PE; psum = [pixels, 100ch]) + bias via K=1 matmul
  softmax over k=25 per parity group (ACT exp, DVE reduce/recip, one
    broadcast tensor_tensor for normalization)
  reassembly: 25-tap per-pixel weighted sum, split across three engine
    channels running in parallel:
      - Pool builds diagonal weight matrices (identity * per-partition
        scalar, 107ns) feeding PE matmuls that accumulate in PSUM
        (107ns each); ACT evacuates PSUM -> bf16 accs
      - DVE runs scalar_tensor_tensor FMA quanta directly (327ns each)
"""

import sys

if "/opt/trn_rl_repo" not in sys.path:
    sys.path.insert(0, "/opt/trn_rl_repo")

import numpy as np

try:
    import ml_dtypes

    BF16 = ml_dtypes.bfloat16
except ImportError:  # pragma: no cover
    BF16 = np.float32

SCALE = 2
K_UP = 5
EPS = 1e-5
B, C, H, W = 2, 256, 64, 64
CM = 64
NK = 100
NCORES = 8
QH = 16           # source rows per core
LH = QH + 4       # local x rows incl halo 2 each side
NW1R = 18         # W1 rows kept: x-local rows [1,19)

# (par, lam) pairs handled by the DVE STT channel; the rest go to the
# pool->PE diagonal-matmul channel.
DVE_PAIRS = [(3, l) for l in range(8)] + [(2, 7)]
POOL_PAIRS = []

_compiled = {}


def _build_nc():
    import concourse.bacc as bacc
    import concourse.bass as bass
    import concourse.mybir as mybir
    import concourse.tile as tile

    f32 = mybir.dt.float32
    bf16 = mybir.dt.bfloat16
    nc = bacc.Bacc("TRN2", target_bir_lowering=False, debug=False)

    # ---- DRAM I/O ----
    x_nat_d = nc.dram_tensor("x_nat", [128, 2, NW1R, W], bf16, kind="ExternalInput")
    xs_d = {}
    for p in range(2):
        for kj in range(K_UP):
            xs_d[(kj, p)] = nc.dram_tensor(
                f"xs_{kj}_{p}", [128, 10, C], bf16, kind="ExternalInput"
            )
    comp_lhsT_d = nc.dram_tensor("comp_lhsT", [128, 2, CM], bf16, kind="ExternalInput")
    comp_bias_d = nc.dram_tensor("comp_bias", [CM, 1], f32, kind="ExternalInput")
    enc_pair_d = nc.dram_tensor("enc_pair", [128, 3, NK], bf16, kind="ExternalInput")
    enc_single_d = nc.dram_tensor("enc_single", [CM, 3, NK], bf16, kind="ExternalInput")
    enc_bias_d = nc.dram_tensor("enc_bias", [1, NK], bf16, kind="ExternalInput")
    rmask_d = nc.dram_tensor("rmask", [CM, NW1R], bf16, kind="ExternalInput")
    iden_d = nc.dram_tensor("iden", [128, 128], bf16, kind="ExternalInput")
    out_d = nc.dram_tensor("out", [4, 128, 8, C], bf16, kind="ExternalOutput")

    taps = [(ki, kj) for ki in (0, 2, 4, 1, 3) for kj in range(K_UP)]
    pe_pairs = [(par, lam) for par in range(4) for lam in range(8)
                if (par, lam) not in DVE_PAIRS]

    with tile.TileContext(nc) as tc:
        with (
            tc.tile_pool(name="consts", bufs=1) as consts,
            tc.tile_pool(name="big", bufs=1) as big,
            tc.tile_pool(name="work", bufs=2) as work,
            tc.tile_pool(name="dgs", bufs=1) as dgs,
        ):
            # ---- load constants; comp weights + x_nat first so the
            # comp->enc->softmax chain starts ASAP ----
            comp_lhsT = consts.tile([128, 2, CM], bf16, tag="comp_lhsT")
            nc.sync.dma_start(comp_lhsT, comp_lhsT_d[:])
            x_nat = big.tile([128, 2, NW1R, W], bf16, tag="x_nat")
            nc.sync.dma_start(x_nat[:, 0], x_nat_d[:, 0])
            nc.sync.dma_start(x_nat[:, 1], x_nat_d[:, 1])
            comp_bias = consts.tile([CM, 1], f32, tag="comp_bias")
            nc.sync.dma_start(comp_bias, comp_bias_d[:])
            rmask = consts.tile([CM, NW1R], bf16, tag="rmask")
            nc.sync.dma_start(rmask, rmask_d[:])
            enc_pair = consts.tile([128, 3, NK], bf16, tag="enc_pair")
            nc.sync.dma_start(enc_pair, enc_pair_d[:])
            enc_single = consts.tile([CM, 3, NK], bf16, tag="enc_single")
            nc.sync.dma_start(enc_single, enc_single_d[:])
            enc_bias = consts.tile([1, NK], bf16, tag="enc_bias")
            nc.sync.dma_start(enc_bias, enc_bias_d[:])
            iden = consts.tile([128, 128], bf16, tag="iden")
            nc.sync.dma_start(iden, iden_d[:])
            ones_row = consts.tile([1, 128], bf16, tag="ones_row")
            nc.vector.memset(ones_row, 1.0)
            # xs in tap-consumption order: p=0 tensors (ki even) first
            xs_t = {}
            for p in range(2):
                for kj in range(K_UP):
                    t = big.tile([128, 10, C], bf16, tag=f"xs_{kj}_{p}")
                    nc.sync.dma_start(t, xs_d[(kj, p)][:])
                    xs_t[(kj, p)] = t

            # ---- comp 1x1 conv: psum[64, 18*64] = comp_w' @ x ----
            pcomp = tc.alloc_tile_pool(name="pcomp", bufs=1, space="PSUM")
            penc = tc.alloc_tile_pool(name="penc", bufs=2, space="PSUM")  # 2 slots: pipeline lam
            # keep-warm junk matmuls: hold the PE busy through the pstate
            # ramp window so the real convs run at full clock
            pwarm = pcomp.tile([CM, 64], f32, tag="pwarm")
            for j in range(60):
                nc.tensor.matmul(
                    pwarm,
                    lhsT=comp_lhsT[:, 0, :],
                    rhs=comp_lhsT[:, 0, 0:64],
                    start=(j == 0),
                    stop=(j == 59),
                )
            psum_c = pcomp.tile([CM, NW1R * W], f32, tag="psum_c")
            nchunks = [(0, 512), (512, 512), (1024, NW1R * W - 1024)]
            x_nat_f = x_nat.rearrange("p h r w -> p h (r w)")
            for h in range(2):
                for n0, nl in nchunks:
                    nc.tensor.matmul(
                        psum_c[:, n0 : n0 + nl],
                        lhsT=comp_lhsT[:, h, :],
                        rhs=x_nat_f[:, h, n0 : n0 + nl],
                        start=(h == 0),
                        stop=(h == 1),
                    )

            # ---- W1 (bf16) + row mask ----
            w1c = big.tile([CM, NW1R, W], bf16, tag="w1c")
            psum_c_v = psum_c.rearrange("p (r w) -> p r w", w=W)
            nc.scalar.activation(
                out=w1c,
                in_=psum_c_v,
                func=mybir.ActivationFunctionType.Relu,
                bias=comp_bias,
                scale=1.0,
            )
            rmask_b = bass.AP(
                tensor=rmask.tensor,
                offset=rmask.offset,
                ap=[list(rmask.ap[0]), list(rmask.ap[1]), [0, W]],
            )
            nc.gpsimd.tensor_tensor(
                out=w1c, in0=w1c, in1=rmask_b, op=mybir.AluOpType.mult
            )
            # w1main: rows 0-63 = W1 shifted right (left tap), rows 64-127 = center
            w1main = big.tile([128, NW1R, W], bf16, tag="w1main")
            nc.vector.memset(w1main[0:CM, :, 0:1], 0.0)
            nc.vector.tensor_copy(out=w1main[CM:128], in_=w1c)
            nc.vector.tensor_copy(
                out=w1main[0:CM, :, 1:W], in_=w1c[:, :, 0 : W - 1]
            )
            # w1sing: W1 shifted left (right tap)
            w1sing = big.tile([CM, NW1R, W], bf16, tag="w1sing")
            nc.vector.memset(w1sing[:, :, W - 1 : W], 0.0)
            nc.vector.tensor_copy(
                out=w1sing[:, :, 0 : W - 1], in_=w1c[:, :, 1:W]
            )
            w1main_f = w1main.rearrange("p r w -> p (r w)")
            w1sing_f = w1sing.rearrange("p r w -> p (r w)")

            # ---- enc 3x3 conv (transposed) + exp; the softmax division
            # is deferred into accumulator normalization ----
            E32 = big.tile([128, 8, NK], f32, tag="E32")
            S = work.tile([128, 8, 4], f32, tag="S")
            R = work.tile([128, 8, 4], f32, tag="R")
            E_v = E32.rearrange("p l (k q) -> p l q k", q=4)
            for lam in range(8):
                psum_e = penc.tile([128, NK], f32, tag="psum_e")
                for ty in range(3):
                    o = (2 * lam + ty) * W
                    nc.tensor.matmul(
                        psum_e,
                        lhsT=w1main_f[:, o : o + 2 * W],
                        rhs=enc_pair[:, ty, :],
                        start=(ty == 0),
                        stop=False,
                    )
                for ty in range(3):
                    o = (2 * lam + ty) * W
                    nc.tensor.matmul(
                        psum_e,
                        lhsT=w1sing_f[:, o : o + 2 * W],
                        rhs=enc_single[:, ty, :],
                        start=False,
                        stop=False,
                    )
                nc.tensor.matmul(
                    psum_e, lhsT=ones_row, rhs=enc_bias, start=False, stop=True
                )
                nc.scalar.activation(
                    out=E32[:, lam, :],
                    in_=psum_e,
                    func=mybir.ActivationFunctionType.Exp,
                )
                nc.vector.tensor_reduce(
                    out=S[:, lam], in_=E_v[:, lam],
                    axis=mybir.AxisListType.X, op=mybir.AluOpType.add,
                )
                nc.vector.reciprocal(out=R[:, lam], in_=S[:, lam])

            penc.release()
            pcomp.release()
            preass = tc.alloc_tile_pool(name="preass", bufs=1, space="PSUM")

            # ---- reassembly ----
            accs = [
                big.tile([128, 8, C], bf16, tag=f"acc{par}", name=f"acc{par}")
                for par in range(4)
            ]
            prods = [
                work.tile([128, C], bf16, tag=f"prod{j}", name=f"prod{j}")
                for j in range(4)
            ]

            def tap_in0(t, lam):
                ki, kj = taps[t]
                return xs_t[(kj, ki % 2)][:, lam + ki // 2, :]

            def tap_ch(t, par):
                ki, kj = taps[t]
                return (ki * K_UP + kj) * 4 + par

            # DVE channel: pairs (par=3, lam 0..7), tap-major; roughly
            # every other tap is ACT-assisted (ACT does the scale-mult,
            # DVE only the 2x bf16 add)
            dve_ops = []
            chan_pairs = ([("pool", p) for p in POOL_PAIRS]
                          + [("dve", p) for p in DVE_PAIRS])
            blk = 3
            for b0 in range(0, len(chan_pairs), blk):
                block = chan_pairs[b0 : b0 + blk]
                for t in range(len(taps)):
                    for kind, (par, lam) in block:
                        dve_ops.append((kind, t, par, lam))

            # PE channel: lam-major pair order so lam-0 quanta unblock
            # right after lam-0 softmax; waves of 8 psum slots
            pe_pair_list = [(par, lam) for lam in range(8) for par in range(4)
                            if (par, lam) not in DVE_PAIRS
                            and (par, lam) not in POOL_PAIRS]
            pe_ops = []
            wv = 6
            for w0 in range(0, len(pe_pair_list), wv):
                wave = pe_pair_list[w0 : w0 + wv]
                for t in range(len(taps)):
                    for wi, (par, lam) in enumerate(wave):
                        pe_ops.append(("mm", t, par, lam, w0 + wi))
                        if t == len(taps) - 1:
                            pe_ops.append(("evac", par, lam, w0 + wi))

            psum_tiles = {}
            diag_i = 0
            prod_i = 0
            di, pi = 0, 0
            while di < len(dve_ops) or pi < len(pe_ops):
                for _ in range(4):
                    if pi < len(pe_ops):
                        op = pe_ops[pi]
                        pi += 1
                        if op[0] == "mm":
                            _, t, par, lam, slot = op
                            if t == 0:
                                psum_tiles[slot] = preass.tile(
                                    [128, C], f32, tag=f"pr{slot % 8}",
                                    name=f"pr{slot % 8}",
                                )
                            dg = dgs.tile(
                                [128, 128], bf16, tag=f"dg{diag_i % 16}",
                                name=f"dg{diag_i % 16}",
                            )
                            diag_i += 1
                            ch = tap_ch(t, par)
                            nc.gpsimd.tensor_scalar_mul(
                                dg, iden, E32[:, lam, ch : ch + 1]
                            )
                            nc.tensor.matmul(
                                psum_tiles[slot],
                                lhsT=dg,
                                rhs=tap_in0(t, lam),
                                start=(t == 0),
                                stop=(t == len(taps) - 1),
                            )
                        else:
                            _, par, lam, slot = op
                            if slot % 3 != 2:
                                nc.scalar.activation(
                                    out=accs[par][:, lam, :],
                                    in_=psum_tiles[slot],
                                    func=mybir.ActivationFunctionType.Copy,
                                    scale=R[:, lam, par : par + 1],
                                )
                            else:
                                nc.vector.tensor_scalar_mul(
                                    accs[par][:, lam, :],
                                    psum_tiles[slot],
                                    R[:, lam, par : par + 1],
                                )
                            nc.sync.dma_start(
                                out_d[par, :, lam], accs[par][:, lam, :]
                            )
                if di < len(dve_ops):
                    kind, t, par, lam = dve_ops[di]
                    di += 1
                    in0 = tap_in0(t, lam)
                    ch = tap_ch(t, par)
                    acc = accs[par][:, lam, :]
                    sc = E32[:, lam, ch : ch + 1]
                    if kind == "pool":
                        nc.gpsimd.scalar_tensor_tensor(
                            out=acc,
                            in0=in0,
                            scalar=sc,
                            in1=in0 if t == 0 else acc,
                            op0=mybir.AluOpType.mult,
                            op1=mybir.AluOpType.bypass
                            if t == 0
                            else mybir.AluOpType.add,
                        )
                        if t == len(taps) - 1:
                            nc.sync.dma_start(
                                out_d[par, :, lam], accs[par][:, lam, :]
                            )
                        continue
                    if t > 0 and (t % 2 == 0 or t in (3, 9)):
                        # ACT-assisted: prod = w * x on ACT, acc += prod on DVE
                        prod = prods[prod_i % 4]
                        prod_i += 1
                        nc.scalar.activation(
                            out=prod, in_=in0,
                            func=mybir.ActivationFunctionType.Copy, scale=sc,
                        )
                        nc.vector.tensor_tensor(
                            out=acc, in0=acc, in1=prod, op=mybir.AluOpType.add
                        )
                    else:
                        nc.vector.scalar_tensor_tensor(
                            out=acc,
                            in0=in0,
                            scalar=sc,
                            in1=in0 if t == 0 else acc,
                            op0=mybir.AluOpType.mult,
                            op1=mybir.AluOpType.bypass
                            if t == 0
                            else mybir.AluOpType.add,
                        )
                    if t == len(taps) - 1:
                        nc.vector.tensor_scalar_mul(
                            accs[par][:, lam, :], accs[par][:, lam, :],
                            R[:, lam, par : par + 1],
                        )
                        nc.sync.dma_start(
                            out_d[par, :, lam], accs[par][:, lam, :]
                        )

            preass.release()

    nc.compile()
    return nc


def _host_inputs(x, comp_w, comp_gamma, comp_beta, comp_mean, comp_var,
                 enc_w, enc_gamma, enc_beta, enc_mean, enc_var):
    """Build the per-core input maps."""
    x = np.asarray(x, dtype=np.float32)
    scale_c = (np.asarray(comp_gamma) / np.sqrt(np.asarray(comp_var) + EPS)).astype(
        np.float32
    )
    bias_c = (np.asarray(comp_beta) - np.asarray(comp_mean) * scale_c).astype(
        np.float32
    )
    wp = np.asarray(comp_w)[:, :, 0, 0].astype(np.float32) * scale_c[:, None]  # [64,256]
    comp_lhsT = np.ascontiguousarray(
        wp.T.reshape(2, 128, CM).transpose(1, 0, 2)
    ).astype(BF16)
    comp_bias = bias_c.reshape(CM, 1)

    scale_e = (np.asarray(enc_gamma) / np.sqrt(np.asarray(enc_var) + EPS)).astype(
        np.float32
    )
    bias_e = (np.asarray(enc_beta) - np.asarray(enc_mean) * scale_e).astype(np.float32)
    ew = np.asarray(enc_w).astype(np.float32) * scale_e[:, None, None, None]  # [100,64,3,3]
    enc_pair = np.zeros((128, 3, NK), np.float32)
    enc_single = np.zeros((CM, 3, NK), np.float32)
    for ty in range(3):
        enc_pair[0:CM, ty] = ew[:, :, ty, 0].T
        enc_pair[CM:128, ty] = ew[:, :, ty, 1].T
        enc_single[:, ty] = ew[:, :, ty, 2].T
    enc_bias_a = bias_e.reshape(1, NK).astype(BF16)

    iden = np.eye(128, dtype=np.float32).astype(BF16)

    in_maps = []
    for core in range(NCORES):
        b, q = core // 4, core % 4
        # padded x slab: rows global [16q-2, 16q+19) -> 21 rows, cols padded +2
        xpad = np.zeros((C, LH + 1, W + 4), np.float32)
        g0, g1 = 16 * q - 2, 16 * q + 19
        s0, s1 = max(g0, 0), min(g1, H)
        xpad[:, s0 - g0 : s1 - g0, 2 : W + 2] = x[b, :, s0:s1, :]

        x_nat = np.ascontiguousarray(
            xpad[:, 1 : 1 + NW1R, 2 : W + 2].reshape(2, 128, NW1R, W).transpose(
                1, 0, 2, 3
            )
        ).astype(BF16)

        m = {"x_nat": x_nat, "comp_lhsT": comp_lhsT, "comp_bias": comp_bias,
             "enc_pair": enc_pair.astype(BF16),
             "enc_single": enc_single.astype(BF16),
             "enc_bias": enc_bias_a, "iden": iden}
        for kj in range(K_UP):
            for p in range(2):
                xv = xpad[:, p : p + 20, kj : kj + W]  # [C, 20, 64]
                arr = xv.reshape(C, 10, 2, W).transpose(2, 3, 1, 0)  # [2,64,10,C]
                m[f"xs_{kj}_{p}"] = np.ascontiguousarray(
                    arr.reshape(128, 10, C)
                ).astype(BF16)
        ridx = np.arange(NW1R)
        grows = 16 * q - 1 + ridx
        m["rmask"] = np.ascontiguousarray(
            np.broadcast_to(
                ((grows >= 0) & (grows < H)).astype(np.float32), (CM, NW1R)
            )
        ).astype(BF16)
        in_maps.append(m)
    return in_maps


def _assemble(results):
    out = np.zeros((B, C, H * SCALE, W * SCALE), np.float32)
    for core in range(NCORES):
        b, q = core // 4, core % 4
        arr = np.asarray(results[core]["out"]).astype(np.float32)  # [4,128,8,256]
        a = arr.reshape(2, 2, 2, 64, 8, C)  # [di, dj, rho, j, lam, c]
        blk = a.transpose(5, 4, 2, 0, 3, 1).reshape(C, 32, 128)
        out[b, :, 32 * q : 32 * q + 32, :] = blk
    return out


def kernel(**inputs):
    from concourse.bass_utils import run_bass_kernel_spmd

    if "nc" not in _compiled:
        _compiled["nc"] = _build_nc()
    nc = _compiled["nc"]
    in_maps = _host_inputs(**inputs)
    res = run_bass_kernel_spmd(nc, in_maps, core_ids=list(range(NCORES)))
    return _assemble(res.results)


# revision 62
# speedup vs baseline: 1.0195x; 1.0152x over previous
"""CARAFE kernel for 8x Trainium2 NeuronCores.

Sharding: core = b*4 + q  (b in [0,2) batch, q in [0,4) H-slice).
Each core handles source rows [16q, 16q+16) of batch b and produces
output rows [32q, 32q+32) (x2 upsampling).

Pipeline per core (bf16 compute, fp32 PSUM/softmax stats):
  comp 1x1 conv (PE, BN folded, ReLU on ACT)
  enc 3x3 conv transposed (PE; psum = [pixels, 100ch]) + bias via K=1 matmul
  softmax over k=25 per parity group (ACT exp, DVE reduce/recip, one
    broadcast tensor_tensor for normalization)
  reassembly: 25-tap per-pixel weighted sum, split across three engine
    channels running in parallel:
      - Pool builds diagonal weight matrices (identity * per-partition
        scalar, 107ns) feeding PE matmuls that accumulate in PSUM
        (107ns each); ACT evacuates PSUM -> bf16 accs
      - DVE runs scalar_tensor_tensor FMA quanta directly (327ns each)
"""

import sys

if "/opt/trn_rl_repo" not in sys.path:
    sys.path.insert(0, "/opt/trn_rl_repo")

import numpy as np

try:
    import ml_dtypes

    BF16 = ml_dtypes.bfloat16
except ImportError:  # pragma: no cover
    BF16 = np.float32

SCALE = 2
K_UP = 5
EPS = 1e-5
B, C, H, W = 2, 256, 64, 64
CM = 64
NK = 100
NCORES = 8
QH = 16           # source rows per core
LH = QH + 4       # local x rows incl halo 2 each side
NW1R = 18         # W1 rows kept: x-local rows [1,19)

# (par, lam) pairs handled by the DVE STT channel; the rest go to the
# pool->PE diagonal-matmul channel.
DVE_PAIRS = [(3, l) for l in range(8)] + [(2, 7)]
POOL_PAIRS = []

_compiled = {}


def _build_nc():
    import concourse.bacc as bacc
    import concourse.bass as bass
    import concourse.mybir as mybir
    import concourse.tile as tile

    f32 = mybir.dt.float32
    bf16 = mybir.dt.bfloat16
    nc = bacc.Bacc("TRN2", target_bir_lowering=False, debug=False)

    # ---- DRAM I/O ----
    x_nat_d = nc.dram_tensor("x_nat", [128, 2, NW1R, W], bf16, kind="ExternalInput")
    xs_d = {}
    for p in range(2):
        for kj in range(K_UP):
            xs_d[(kj, p)] = nc.dram_tensor(
                f"xs_{kj}_{p}", [128, 10, C], bf16, kind="ExternalInput"
            )
    comp_lhsT_d = nc.dram_tensor("comp_lhsT", [128, 2, CM], bf16, kind="ExternalInput")
    comp_bias_d = nc.dram_tensor("comp_bias", [CM, 1], f32, kind="ExternalInput")
    enc_pair_d = nc.dram_tensor("enc_pair", [128, 3, NK], bf16, kind="ExternalInput")
    enc_single_d = nc.dram_tensor("enc_single", [CM, 3, NK], bf16, kind="ExternalInput")
    enc_bias_d = nc.dram_tensor("enc_bias", [1, NK], bf16, kind="ExternalInput")
    rmask_d = nc.dram_tensor("rmask", [CM, NW1R], bf16, kind="ExternalInput")
    iden_d = nc.dram_tensor("iden", [128, 128], bf16, kind="ExternalInput")
    out_d = nc.dram_tensor("out", [4, 128, 8, C], bf16, kind="ExternalOutput")

    taps = [(ki, kj) for ki in (0, 2, 4, 1, 3) for kj in range(K_UP)]
    pe_pairs = [(par, lam) for par in range(4) for lam in range(8)
                if (par, lam) not in DVE_PAIRS]

    with tile.TileContext(nc) as tc:
        with (
            tc.tile_pool(name="consts", bufs=1) as consts,
            tc.tile_pool(name="big", bufs=1) as big,
            tc.tile_pool(name="work", bufs=2) as work,
            tc.tile_pool(name="dgs", bufs=1) as dgs,
        ):
            # ---- load constants; comp weights + x_nat first so the
            # comp->enc->softmax chain starts ASAP ----
            comp_lhsT = consts.tile([128, 2, CM], bf16, tag="comp_lhsT")
            nc.sync.dma_start(comp_lhsT, comp_lhsT_d[:])
            x_nat = big.tile([128, 2, NW1R, W], bf16, tag="x_nat")
            nc.sync.dma_start(x_nat[:, 0], x_nat_d[:, 0])
            nc.sync.dma_start(x_nat[:, 1], x_nat_d[:, 1])
            comp_bias = consts.tile([CM, 1], f32, tag="comp_bias")
            nc.sync.dma_start(comp_bias, comp_bias_d[:])
            rmask = consts.tile([CM, NW1R], bf16, tag="rmask")
            nc.sync.dma_start(rmask, rmask_d[:])
            enc_pair = consts.tile([128, 3, NK], bf16, tag="enc_pair")
            nc.sync.dma_start(enc_pair, enc_pair_d[:])
            enc_single = consts.tile([CM, 3, NK], bf16, tag="enc_single")
            nc.sync.dma_start(enc_single, enc_single_d[:])
            enc_bias = consts.tile([1, NK], bf16, tag="enc_bias")
            nc.sync.dma_start(enc_bias, enc_bias_d[:])
            iden = consts.tile([128, 128], bf16, tag="iden")
            nc.sync.dma_start(iden, iden_d[:])
            ones_row = consts.tile([1, 128], bf16, tag="ones_row")
            nc.vector.memset(ones_row, 1.0)
            # xs in tap-consumption order: p=0 tensors (ki even) first
            xs_t = {}
            for p in range(2):
                for kj in range(K_UP):
                    t = big.tile([128, 10, C], bf16, tag=f"xs_{kj}_{p}")
                    nc.sync.dma_start(t, xs_d[(kj, p)][:])
                    xs_t[(kj, p)] = t

            # ---- comp 1x1 conv: psum[64, 18*64] = comp_w' @ x ----
            pcomp = tc.alloc_tile_pool(name="pcomp", bufs=1, space="PSUM")
            penc = tc.alloc_tile_pool(name="penc", bufs=2, space="PSUM")  # 2 slots: pipeline lam
            # keep-warm junk matmuls: hold the PE busy through the pstate
            # ramp window so the real convs run at full clock
            pwarm = pcomp.tile([CM, 64], f32, tag="pwarm")
            for j in range(60):
                nc.tensor.matmul(
                    pwarm,
                    lhsT=comp_lhsT[:, 0, :],
                    rhs=comp_lhsT[:, 0, 0:64],
                    start=(j == 0),
                    stop=(j == 59),
                )
            psum_c = pcomp.tile([CM, NW1R * W], f32, tag="psum_c")
            nchunks = [(0, 512), (512, 512), (1024, NW1R * W - 1024)]
            x_nat_f = x_nat.rearrange("p h r w -> p h (r w)")
            for h in range(2):
                for n0, nl in nchunks:
                    nc.tensor.matmul(
                        psum_c[:, n0 : n0 + nl],
                        lhsT=comp_lhsT[:, h, :],
                        rhs=x_nat_f[:, h, n0 : n0 + nl],
                        start=(h == 0),
                        stop=(h == 1),
                    )

            # ---- W1 (bf16) + row mask ----
            w1c = big.tile([CM, NW1R, W], bf16, tag="w1c")
            psum_c_v = psum_c.rearrange("p (r w) -> p r w", w=W)
            nc.scalar.activation(
                out=w1c,
                in_=psum_c_v,
                func=mybir.ActivationFunctionType.Relu,
                bias=comp_bias,
                scale=1.0,
            )
            rmask_b = bass.AP(
                tensor=rmask.tensor,
                offset=rmask.offset,
                ap=[list(rmask.ap[0]), list(rmask.ap[1]), [0, W]],
            )
            nc.gpsimd.tensor_tensor(
                out=w1c, in0=w1c, in1=rmask_b, op=mybir.AluOpType.mult
            )
            # w1main: rows 0-63 = W1 shifted right (left tap), rows 64-127 = center
            w1main = big.tile([128, NW1R, W], bf16, tag="w1main")
            nc.vector.memset(w1main[0:CM, :, 0:1], 0.0)
            nc.vector.tensor_copy(out=w1main[CM:128], in_=w1c)
            nc.vector.tensor_copy(
                out=w1main[0:CM, :, 1:W], in_=w1c[:, :, 0 : W - 1]
            )
            # w1sing: W1 shifted left (right tap)
            w1sing = big.tile([CM, NW1R, W], bf16, tag="w1sing")
            nc.vector.memset(w1sing[:, :, W - 1 : W], 0.0)
            nc.vector.tensor_copy(
                out=w1sing[:, :, 0 : W - 1], in_=w1c[:, :, 1:W]
            )
            w1main_f = w1main.rearrange("p r w -> p (r w)")
            w1sing_f = w1sing.rearrange("p r w -> p (r w)")

            # ---- enc 3x3 conv (transposed) + exp; the softmax division
            # is deferred into accumulator normalization ----
            E32 = big.tile([128, 8, NK], f32, tag="E32")
            S = work.tile([128, 8, 4], f32, tag="S")
            R = work.tile([128, 8, 4], f32, tag="R")
            E_v = E32.rearrange("p l (k q) -> p l q k", q=4)
            for lam in range(8):
                psum_e = penc.tile([128, NK], f32, tag="psum_e")
                for ty in range(3):
                    o = (2 * lam + ty) * W
                    nc.tensor.matmul(
                        psum_e,
                        lhsT=w1main_f[:, o : o + 2 * W],
                        rhs=enc_pair[:, ty, :],
                        start=(ty == 0),
                        stop=False,
                    )
                for ty in range(3):
                    o = (2 * lam + ty) * W
                    nc.tensor.matmul(
                        psum_e,
                        lhsT=w1sing_f[:, o : o + 2 * W],
                        rhs=enc_single[:, ty, :],
                        start=False,
                        stop=False,
                    )
                nc.tensor.matmul(
                    psum_e, lhsT=ones_row, rhs=enc_bias, start=False, stop=True
                )
                nc.scalar.activation(
                    out=E32[:, lam, :],
                    in_=psum_e,
                    func=mybir.ActivationFunctionType.Exp,
                )
                nc.vector.tensor_reduce(
                    out=S[:, lam], in_=E_v[:, lam],
                    axis=mybir.AxisListType.X, op=mybir.AluOpType.add,
                )
                nc.vector.reciprocal(out=R[:, lam], in_=S[:, lam])

            penc.release()
            pcomp.release()
            preass = tc.alloc_tile_pool(name="preass", bufs=1, space="PSUM")

            # ---- reassembly ----
            accs = [
                big.tile([128, 8, C], bf16, tag=f"acc{par}", name=f"acc{par}")
                for par in range(4)
            ]
            prods = [
                work.tile([128, C], bf16, tag=f"prod{j}", name=f"prod{j}")
                for j in range(4)
            ]

            def tap_in0(t, lam):
                ki, kj = taps[t]
                return xs_t[(kj, ki % 2)][:, lam + ki // 2, :]

            def tap_ch(t, par):
                ki, kj = taps[t]
                return (ki * K_UP + kj) * 4 + par

            # DVE channel: pairs (par=3, lam 0..7), tap-major; roughly
            # every other tap is ACT-assisted (ACT does the scale-mult,
            # DVE only the 2x bf16 add)
            dve_ops = []
            chan_pairs = ([("pool", p) for p in POOL_PAIRS]
                          + [("dve", p) for p in DVE_PAIRS])
            blk = 3
            for b0 in range(0, len(chan_pairs), blk):
                block = chan_pairs[b0 : b0 + blk]
                for t in range(len(taps)):
                    for kind, (par, lam) in block:
                        dve_ops.append((kind, t, par, lam))

            # PE channel: lam-major pair order so lam-0 quanta unblock
            # right after lam-0 softmax; waves of 8 psum slots
            pe_pair_list = [(par, lam) for lam in range(8) for par in range(4)
                            if (par, lam) not in DVE_PAIRS
                            and (par, lam) not in POOL_PAIRS]
            pe_ops = []
            wv = 7
            for w0 in range(0, len(pe_pair_list), wv):
                wave = pe_pair_list[w0 : w0 + wv]
                for t in range(len(taps)):
                    for wi, (par, lam) in enumerate(wave):
                        pe_ops.append(("mm", t, par, lam, w0 + wi))
                        if t == len(taps) - 1:
                            pe_ops.append(("evac", par, lam, w0 + wi))

            psum_tiles = {}
            diag_i = 0
            prod_i = 0
            di, pi = 0, 0
            while di < len(dve_ops) or pi < len(pe_ops):
                for _ in range(4):
                    if pi < len(pe_ops):
                        op = pe_ops[pi]
                        pi += 1
                        if op[0] == "mm":
                            _, t, par, lam, slot = op
                            if t == 0:
                                psum_tiles[slot] = preass.tile(
                                    [128, C], f32, tag=f"pr{slot % 8}",
                                    name=f"pr{slot % 8}",
                                )
                            dg = dgs.tile(
                                [128, 128], bf16, tag=f"dg{diag_i % 16}",
                                name=f"dg{diag_i % 16}",
                            )
                            diag_i += 1
                            ch = tap_ch(t, par)
                            nc.gpsimd.tensor_scalar_mul(
                                dg, iden, E32[:, lam, ch : ch + 1]
                            )
                            nc.tensor.matmul(
                                psum_tiles[slot],
                                lhsT=dg,
                                rhs=tap_in0(t, lam),
                                start=(t == 0),
                                stop=(t == len(taps) - 1),
                            )
                        else:
                            _, par, lam, slot = op
                            if slot % 3 != 2:
                                nc.scalar.activation(
                                    out=accs[par][:, lam, :],
                                    in_=psum_tiles[slot],
                                    func=mybir.ActivationFunctionType.Copy,
                                    scale=R[:, lam, par : par + 1],
                                )
                            else:
                                nc.vector.tensor_scalar_mul(
                                    accs[par][:, lam, :],
                                    psum_tiles[slot],
                                    R[:, lam, par : par + 1],
                                )
                            nc.sync.dma_start(
                                out_d[par, :, lam], accs[par][:, lam, :]
                            )
                if di < len(dve_ops):
                    kind, t, par, lam = dve_ops[di]
                    di += 1
                    in0 = tap_in0(t, lam)
                    ch = tap_ch(t, par)
                    acc = accs[par][:, lam, :]
                    sc = E32[:, lam, ch : ch + 1]
                    if kind == "pool":
                        nc.gpsimd.scalar_tensor_tensor(
                            out=acc,
                            in0=in0,
                            scalar=sc,
                            in1=in0 if t == 0 else acc,
                            op0=mybir.AluOpType.mult,
                            op1=mybir.AluOpType.bypass
                            if t == 0
                            else mybir.AluOpType.add,
                        )
                        if t == len(taps) - 1:
                            nc.sync.dma_start(
                                out_d[par, :, lam], accs[par][:, lam, :]
                            )
                        continue
                    if t > 0 and (t % 2 == 0 or t in (3, 9)):
                        # ACT-assisted: prod = w * x on ACT, acc += prod on DVE
                        prod = prods[prod_i % 4]
                        prod_i += 1
                        nc.scalar.activation(
                            out=prod, in_=in0,
                            func=mybir.ActivationFunctionType.Copy, scale=sc,
                        )
                        nc.vector.tensor_tensor(
                            out=acc, in0=acc, in1=prod, op=mybir.AluOpType.add
                        )
                    else:
                        nc.vector.scalar_tensor_tensor(
                            out=acc,
                            in0=in0,
                            scalar=sc,
                            in1=in0 if t == 0 else acc,
                            op0=mybir.AluOpType.mult,
                            op1=mybir.AluOpType.bypass
                            if t == 0
                            else mybir.AluOpType.add,
                        )
                    if t == len(taps) - 1:
                        nc.vector.tensor_scalar_mul(
                            accs[par][:, lam, :], accs[par][:, lam, :],
                            R[:, lam, par : par + 1],
                        )
                        nc.sync.dma_start(
                            out_d[par, :, lam], accs[par][:, lam, :]
                        )

            preass.release()

    nc.compile()
    return nc


def _host_inputs(x, comp_w, comp_gamma, comp_beta, comp_mean, comp_var,
                 enc_w, enc_gamma, enc_beta, enc_mean, enc_var):
    """Build the per-core input maps."""
    x = np.asarray(x, dtype=np.float32)
    scale_c = (np.asarray(comp_gamma) / np.sqrt(np.asarray(comp_var) + EPS)).astype(
        np.float32
    )
    bias_c = (np.asarray(comp_beta) - np.asarray(comp_mean) * scale_c).astype(
        np.float32
    )
    wp = np.asarray(comp_w)[:, :, 0, 0].astype(np.float32) * scale_c[:, None]  # [64,256]
    comp_lhsT = np.ascontiguousarray(
        wp.T.reshape(2, 128, CM).transpose(1, 0, 2)
    ).astype(BF16)
    comp_bias = bias_c.reshape(CM, 1)

    scale_e = (np.asarray(enc_gamma) / np.sqrt(np.asarray(enc_var) + EPS)).astype(
        np.float32
    )
    bias_e = (np.asarray(enc_beta) - np.asarray(enc_mean) * scale_e).astype(np.float32)
    ew = np.asarray(enc_w).astype(np.float32) * scale_e[:, None, None, None]  # [100,64,3,3]
    enc_pair = np.zeros((128, 3, NK), np.float32)
    enc_single = np.zeros((CM, 3, NK), np.float32)
    for ty in range(3):
        enc_pair[0:CM, ty] = ew[:, :, ty, 0].T
        enc_pair[CM:128, ty] = ew[:, :, ty, 1].T
        enc_single[:, ty] = ew[:, :, ty, 2].T
    enc_bias_a = bias_e.reshape(1, NK).astype(BF16)

    iden = np.eye(128, dtype=np.float32).astype(BF16)

    in_maps = []
    for core in range(NCORES):
        b, q = core // 4, core % 4
        # padded x slab: rows global [16q-2, 16q+19) -> 21 rows, cols padded +2
        xpad = np.zeros((C, LH + 1, W + 4), np.float32)
        g0, g1 = 16 * q - 2, 16 * q + 19
        s0, s1 = max(g0, 0), min(g1, H)
        xpad[:, s0 - g0 : s1 - g0, 2 : W + 2] = x[b, :, s0:s1, :]

        x_nat = np.ascontiguousarray(
            xpad[:, 1 : 1 + NW1R, 2 : W + 2].reshape(2, 128, NW1R, W).transpose(
                1, 0, 2, 3
            )
        ).astype(BF16)

        m = {"x_nat": x_nat, "comp_lhsT": comp_lhsT, "comp_bias": comp_bias,
             "enc_pair": enc_pair.astype(BF16),
             "enc_single": enc_single.astype(BF16),
             "enc_bias": enc_bias_a, "iden": iden}
        for kj in range(K_UP):
            for p in range(2):
                xv = xpad[:, p : p + 20, kj : kj + W]  # [C, 20, 64]
                arr = xv.reshape(C, 10, 2, W).transpose(2, 3, 1, 0)  # [2,64,10,C]
                m[f"xs_{kj}_{p}"] = np.ascontiguousarray(
                    arr.reshape(128, 10, C)
                ).astype(BF16)
        ridx = np.arange(NW1R)
        grows = 16 * q - 1 + ridx
        m["rmask"] = np.ascontiguousarray(
            np.broadcast_to(
                ((grows >= 0) & (grows < H)).astype(np.float32), (CM, NW1R)
            )
        ).astype(BF16)
        in_maps.append(m)
    return in_maps


def _assemble(results):
    out = np.zeros((B, C, H * SCALE, W * SCALE), np.float32)
    for core in range(NCORES):
        b, q = core // 4, core % 4
        arr = np.asarray(results[core]["out"]).astype(np.float32)  # [4,128,8,256]
        a = arr.reshape(2, 2, 2, 64, 8, C)  # [di, dj, rho, j, lam, c]
        blk = a.transpose(5, 4, 2, 0, 3, 1).reshape(C, 32, 128)
        out[b, :, 32 * q : 32 * q + 32, :] = blk
    return out


def kernel(**inputs):
    from concourse.bass_utils import run_bass_kernel_spmd

    if "nc" not in _compiled:
        _compiled["nc"] = _build_nc()
    nc = _compiled["nc"]
    in_maps = _host_inputs(**inputs)
    res = run_bass_kernel_spmd(nc, in_maps, core_ids=list(range(NCORES)))
    return _assemble(res.results)
